# revision 1
# baseline (speedup 1.0000x reference)
"""Multi-head causal attention (B=4, T=2048, E=1024, H=16) on 8 TRN2 NeuronCores.

Sharding: core c handles batch b = c//2 and head-group g = c%2 (8 heads = 512
of the 1024 embedding dims). Each core runs an independent single-core kernel:

  QT = (Wq_g @ xq.T)        [512, T]   (d on partitions, 4 strips of 128)
  KT = (Wk_g @ xkv.T)       [512, T]
  V  = (xkv @ Wv_g.T)       [T, 512]   (t on partitions, + ones column -> VE)
  per (tq-chunk c512, head h):
     S.T[tk_blk j, tq] = KT_h[:, j].T @ QT_h[:, c512]   (K=64 matmul)
     P.T = exp(S.T / 8) * causal_mask                    (ScalarE + DVE)
     O.T[65, 512] += [V_h | 1][tk_blk].T @ P.T           (PSUM accumulate)
     O = transpose(O.T); out = O[:, :64] / O[:, 64]      (PE + DVE)

Inputs are pre-transposed and bf16-cast on the host; matmuls are bf16 with
fp32 PSUM accumulation; softmax runs unnormalized exp (scores are O(1) by
construction) with the denominator from the appended ones column.
"""

import os
import numpy as np
import ml_dtypes

import concourse.bass as bass
import concourse.bacc as bacc
import concourse.mybir as mybir
import concourse.tile as tile
from concourse.bass_utils import run_bass_kernel_spmd
from concourse.masks import make_identity

F32 = mybir.dt.float32
BF16 = mybir.dt.bfloat16

P = 128  # partitions
D = 64  # head dim
B, T_FULL, E, H_TOT = 4, 2048, 1024, 16
HLOC = 8  # heads per core
DLOC = HLOC * D  # 512: local slice of E
N_CORES = 8


def build(T=T_FULL):
    """Single-core graph; same graph runs SPMD on all 8 cores."""
    assert T % 512 == 0
    TC = T // 512  # tq chunks of 512
    NTB = T // P  # tk blocks of 128
    KCH = E // P  # 8 contraction chunks for projections
    MCH = DLOC // P  # 4 output strips for QT/KT

    nc = bacc.Bacc("TRN2", target_bir_lowering=False, debug=False,
                   num_devices=N_CORES)

    xqT = nc.dram_tensor("xqT", [E, T], BF16, kind="ExternalInput")
    xkvT = nc.dram_tensor("xkvT", [E, T], BF16, kind="ExternalInput")
    wqT = nc.dram_tensor("wqT", [E, DLOC], BF16, kind="ExternalInput")
    wkT = nc.dram_tensor("wkT", [E, DLOC], BF16, kind="ExternalInput")
    wvT = nc.dram_tensor("wvT", [E, DLOC], BF16, kind="ExternalInput")
    out = nc.dram_tensor("out", [T, DLOC], F32, kind="ExternalOutput")

    xqT_v = xqT.ap().rearrange("(k p) t -> p k t", p=P)
    xkvT_v = xkvT.ap().rearrange("(k p) t -> p k t", p=P)

    with tile.TileContext(nc) as tc:
        with (
            tc.tile_pool(name="persist", bufs=1) as persist,
            tc.tile_pool(name="wpool", bufs=3) as wpool,
            tc.tile_pool(name="xpool", bufs=3) as xpool,
            tc.tile_pool(name="lpool", bufs=5) as lpool,
            tc.tile_pool(name="ptpool", bufs=26) as ptpool,
            tc.tile_pool(name="otpool", bufs=4) as otpool,
            tc.tile_pool(name="osb", bufs=3) as osb,
            tc.tile_pool(name="rpool", bufs=8) as rpool,
            tc.tile_pool(name="mm_ps", bufs=5, space="PSUM") as mm_ps,
            tc.tile_pool(name="pv_ps", bufs=1, space="PSUM") as pv_ps,
            tc.tile_pool(name="ot_ps", bufs=2, space="PSUM") as ot_ps,
        ):
            # ---- constants ----
            ident = persist.tile([P, P], F32, tag="ident")
            make_identity(nc, ident[:])
            # masks4[:, r, :]: cols [0,128r) = 0, cols [128r,128r+128) =
            # upper triangle (keep col >= row), rest = 1
            masks4 = persist.tile([P, 4, 512], BF16, tag="masks4")
            nc.gpsimd.memset(masks4[:], 1.0)
            for r in range(4):
                if r > 0:
                    nc.gpsimd.memset(masks4[:, r, 0 : P * r], 0.0)
                nc.gpsimd.affine_select(
                    out=masks4[:, r, P * r : P * r + P],
                    in_=masks4[:, r, P * r : P * r + P],
                    compare_op=mybir.AluOpType.is_ge,
                    fill=0.0,
                    base=0,
                    pattern=[[1, P]],
                    channel_multiplier=-1,
                )

            QT = persist.tile([P, MCH, T], BF16, tag="QT")
            KT = persist.tile([P, MCH, T], BF16, tag="KT")
            VE = persist.tile([P, NTB, HLOC, D + 1], BF16, tag="VE")

            # weights resident for all projection slices
            wts = {}
            for nm, wdram in (("q", wqT), ("k", wkT), ("v", wvT)):
                wt = wpool.tile([P, KCH, DLOC], BF16, tag="w", name=f"w{nm}")
                for k in range(KCH):
                    nc.sync.dma_start(
                        wt[:, k, :], wdram.ap()[P * k : P * k + P, :]
                    )
                wts[nm] = wt

            def emit_proj_slice(n, part="all"):
                """QT/KT strips and VE blocks for t in [512n, 512(n+1))."""
                pairs = (("q", QT, xqT_v), ("k", KT, xkvT_v))
                if part == "v":
                    pairs = ()
                for nm, dst, xv in pairs:
                    wt = wts[nm]
                    xt = xpool.tile(
                        [P, KCH, 512], BF16, tag="x", name=f"x{nm}{n}"
                    )
                    for k in range(KCH):
                        nc.sync.dma_start(
                            xt[:, k, :], xv[:, k, 512 * n : 512 * n + 512]
                        )
                    for m in range(MCH):
                        ps = mm_ps.tile([P, 512], F32, tag="s")
                        for k in range(KCH):
                            nc.tensor.matmul(
                                ps[:],
                                wt[:, k, P * m : P * m + P],
                                xt[:, k, :],
                                start=(k == 0),
                                stop=(k == KCH - 1),
                            )
                        nc.vector.tensor_copy(
                            dst[:, m, 512 * n : 512 * n + 512], ps[:]
                        )
                if part == "qk":
                    return
                wv = wts["v"]
                for i in range(4 * n, 4 * n + 4):
                    nc.vector.memset(VE[:, i, :, D : D + 1], 1.0)
                    lt = lpool.tile([P, KCH, P], BF16, tag="l", name=f"l{i}")
                    for k in range(KCH):
                        nc.sync.dma_start(
                            lt[:, k, :], xkvT_v[:, k, P * i : P * i + P]
                        )
                    ps = mm_ps.tile([P, 512], F32, tag="s")
                    for k in range(KCH):
                        nc.tensor.matmul(
                            ps[:],
                            lt[:, k, :],
                            wv[:, k, :],
                            start=(k == 0),
                            stop=(k == KCH - 1),
                        )
                    nc.vector.tensor_copy(
                        VE[:, i, :, 0:D],
                        ps[:].rearrange("p (h d) -> p h d", h=HLOC),
                    )

            # ---- attention, software-pipelined over (c, h), with the
            # next projection slice interleaved mid-chunk so the
            # TensorEngine has work while ScalarE drains exps ----
            osb_tiles = {}

            def emit_qk(c, h):
                """S.T strips + exp + mask for one (tq-chunk, head).

                Diagonal blocks (j >= 4c) only need columns [128r, 512)
                of the tq chunk (r = j - 4c); QK/exp/mask are trimmed to
                that width and PV below accumulates the same subrange.
                """
                s, po = h // 2, D * (h % 2)
                nj = 4 * c + 4
                pts = []
                for j in range(nj):
                    r = j - 4 * c
                    st = P * r if r > 0 else 0
                    sps = mm_ps.tile([P, 512], F32, tag="s")
                    pt = ptpool.tile([P, 512], BF16, tag="pt")
                    nc.tensor.matmul(
                        sps[:, st:512],
                        KT[po : po + D, s, P * j : P * j + P],
                        QT[po : po + D, s, 512 * c + st : 512 * c + 512],
                        start=True,
                        stop=True,
                    )
                    nc.scalar.activation(
                        pt[:, st:512],
                        sps[:, st:512],
                        mybir.ActivationFunctionType.Exp,
                        scale=0.125,
                    )
                    if r >= 0:
                        nc.vector.tensor_mul(
                            pt[:, st:512],
                            pt[:, st:512],
                            masks4[:, r, st:512],
                        )
                    pts.append((pt, st))
                return pts

            def emit_pv(c, h, pts):
                """PV accumulate + epilogue for one (tq-chunk, head)."""
                nj = 4 * c + 4
                pv = pv_ps.tile([D + 1, 512], F32, tag="pv")
                for j in range(nj):
                    pt, st = pts[j]
                    nc.tensor.matmul(
                        pv[:, st:512],
                        VE[:, j, h, :],
                        pt[:, st:512],
                        start=(j == 0),
                        stop=(j == nj - 1),
                    )
                ot = otpool.tile([D + 1, 512], F32, tag="ot")
                nc.vector.tensor_copy(ot[:], pv[:])
                oc = osb_tiles[c]
                for s4 in range(4):
                    tp = ot_ps.tile([P, D + 1], F32, tag="tp")
                    nc.tensor.transpose(
                        tp[:],
                        ot[:, P * s4 : P * s4 + P],
                        ident[0 : D + 1, 0 : D + 1],
                    )
                    r_ = rpool.tile([P, 1], F32, tag="r")
                    nc.vector.reciprocal(r_[:], tp[:, D : D + 1])
                    nc.vector.tensor_scalar_mul(
                        oc[:, s4, D * h : D * h + D], tp[:, 0:D], r_[:]
                    )

            def emit_out_dma(cc):
                for s4 in range(4):
                    nc.sync.dma_start(
                        out.ap()[
                            512 * cc + P * s4 : 512 * cc + P * s4 + P, :
                        ],
                        osb_tiles[cc][:, s4, :],
                    )

            emit_proj_slice(0)
            pending = None
            for c in range(TC):
                osb_tiles[c] = osb.tile(
                    [P, 4, 512], F32, tag="o", name=f"osb{c}"
                )
                for h in range(HLOC):
                    pts = emit_qk(c, h)
                    if pending is not None:
                        emit_pv(*pending)
                        if pending[1] == HLOC - 1:
                            emit_out_dma(pending[0])
                    pending = (c, h, pts)
                    if h == 2 and c + 1 < TC:
                        emit_proj_slice(c + 1, part="qk")
                    if h == 5 and c + 1 < TC:
                        emit_proj_slice(c + 1, part="v")
            emit_pv(*pending)
            emit_out_dma(pending[0])

    nc.compile()
    return nc


_NC_CACHE = {}


def _get_nc(T):
    if T not in _NC_CACHE:
        _NC_CACHE[T] = build(T)
    return _NC_CACHE[T]


def kernel(inputs_q, inputs_kv, Wq, Wk, Wv):
    inputs_q = np.asarray(inputs_q, dtype=np.float32)
    inputs_kv = np.asarray(inputs_kv, dtype=np.float32)
    Wq = np.asarray(Wq, dtype=np.float32)
    Wk = np.asarray(Wk, dtype=np.float32)
    Wv = np.asarray(Wv, dtype=np.float32)
    T = inputs_q.shape[1]

    bf = ml_dtypes.bfloat16
    in_maps = []
    for c in range(N_CORES):
        b, g = c // 2, c % 2
        sl = slice(g * DLOC, (g + 1) * DLOC)
        in_maps.append(
            {
                "xqT": np.ascontiguousarray(inputs_q[b].T).astype(bf),
                "xkvT": np.ascontiguousarray(inputs_kv[b].T).astype(bf),
                "wqT": np.ascontiguousarray(Wq[sl].T).astype(bf),
                "wkT": np.ascontiguousarray(Wk[sl].T).astype(bf),
                "wvT": np.ascontiguousarray(Wv[sl].T).astype(bf),
            }
        )

    nc = _get_nc(T)
    trace = bool(int(os.environ.get("KERNEL_TRACE", "0")))
    res = run_bass_kernel_spmd(
        nc, in_maps, core_ids=list(range(N_CORES)), trace=trace
    )
    if trace:
        kernel.last_result = res

    full = np.empty((B, T, E), np.float32)
    for c in range(N_CORES):
        b, g = c // 2, c % 2
        full[b, :, g * DLOC : (g + 1) * DLOC] = res.results[c]["out"]
    return full



# revision 5
# speedup vs baseline: 1.2176x; 1.2176x over previous
"""Multi-head causal attention (B=4, T=2048, E=1024, H=16) on 8 TRN2 NeuronCores.

Sharding: core c handles batch b = c//2 and head-group g = c%2 (8 heads = 512
of the 1024 embedding dims). Each core runs an independent single-core kernel:

  QT = (Wq_g @ xq.T)        [512, T]   (d on partitions, 4 strips of 128)
  KT = (Wk_g @ xkv.T)       [512, T]
  V  = (xkv @ Wv_g.T)       [T, 512]   (t on partitions, + ones column -> VE)
  per (tq-chunk c512, head h):
     S.T[tk_blk j, tq] = KT_h[:, j].T @ QT_h[:, c512]   (K=64 matmul)
     P.T = exp(S.T / 8) * causal_mask                    (ScalarE + DVE)
     O.T[65, 512] += [V_h | 1][tk_blk].T @ P.T           (PSUM accumulate)
     O = transpose(O.T); out = O[:, :64] / O[:, 64]      (PE + DVE)

Matmuls are bf16 with fp32 PSUM accumulation; softmax runs unnormalized exp
(scores are O(1) by construction) with the denominator from the appended ones
column.

Pipeline structure (v2): QK blocks are emitted in pairs sharing a 2-bank PSUM
tile consumed by a single exp ACTIVATE; only the 128-wide diagonal sub-blocks
are mask-multiplied; PV for step s-1 is interleaved between the QK pairs of
step s so the TensorEngine never waits on the ScalarE exp chain; the
projection of tq-slice c+1 is spread in 8-matmul units across chunk c's steps.
PSUM budget is exactly 8 banks: 2x2 (QK pairs) + 2x1 (proj) + 1 (PV) + 1
(transpose epilogue).
"""

import os
import numpy as np
import ml_dtypes

import concourse.bass as bass
import concourse.bacc as bacc
import concourse.mybir as mybir
import concourse.tile as tile
from concourse.bass_utils import run_bass_kernel_spmd
from concourse.masks import make_identity

F32 = mybir.dt.float32
BF16 = mybir.dt.bfloat16
EXP = mybir.ActivationFunctionType.Exp

P = 128  # partitions
D = 64  # head dim
B, T_FULL, E, H_TOT = 4, 2048, 1024, 16
HLOC = 8  # heads per core
DLOC = HLOC * D  # 512: local slice of E
N_CORES = 8


def build(T=T_FULL):
    """Single-core graph; same graph runs SPMD on all 8 cores."""
    assert T % 512 == 0
    TC = T // 512  # tq chunks of 512
    NTB = T // P  # tk blocks of 128
    KCH = E // P  # 8 contraction chunks for projections
    MCH = DLOC // P  # 4 output strips for QT/KT

    nc = bacc.Bacc("TRN2", target_bir_lowering=False, debug=False,
                   num_devices=N_CORES)

    xqT = nc.dram_tensor("xqT", [E, T], BF16, kind="ExternalInput")
    xkvT = nc.dram_tensor("xkvT", [E, T], BF16, kind="ExternalInput")
    wqT = nc.dram_tensor("wqT", [E, DLOC], BF16, kind="ExternalInput")
    wkT = nc.dram_tensor("wkT", [E, DLOC], BF16, kind="ExternalInput")
    wvT = nc.dram_tensor("wvT", [E, DLOC], BF16, kind="ExternalInput")
    out = nc.dram_tensor("out", [T, DLOC], F32, kind="ExternalOutput")

    xq_v = xqT.ap().rearrange("(k p) t -> p k t", p=P)
    xkv_v = xkvT.ap().rearrange("(k p) t -> p k t", p=P)
    wq_v = wqT.ap().rearrange("(k p) d -> p k d", p=P)
    wk_v = wkT.ap().rearrange("(k p) d -> p k d", p=P)
    wv_v = wvT.ap().rearrange("(k p) d -> p k d", p=P)
    out_v = out.ap().rearrange("(c s p) d -> c p s d", p=P, s=4)

    with tile.TileContext(nc) as tc:
        with (
            tc.tile_pool(name="persist", bufs=1) as persist,
            tc.tile_pool(name="xqpool", bufs=2) as xqpool,
            tc.tile_pool(name="xkpool", bufs=2) as xkpool,
            tc.tile_pool(name="ptpool", bufs=16) as ptpool,
            tc.tile_pool(name="otpool", bufs=2) as otpool,
            tc.tile_pool(name="osb", bufs=2) as osb,
            tc.tile_pool(name="rpool", bufs=2) as rpool,
            tc.tile_pool(name="qk_ps", bufs=2, space="PSUM") as qk_ps,
            tc.tile_pool(name="pj_ps", bufs=2, space="PSUM") as pj_ps,
            tc.tile_pool(name="pv_ps", bufs=1, space="PSUM") as pv_ps,
            tc.tile_pool(name="tp_ps", bufs=1, space="PSUM") as tp_ps,
        ):
            # ---- constants ----
            ident = persist.tile([P, P], F32, tag="ident")
            make_identity(nc, ident[:])
            # tri2[:, a, :]: upper triangle (keep col >= row), for the two
            # 128-wide diagonal sub-blocks handled per mask op
            tri2 = persist.tile([P, 2, P], BF16, tag="tri2")
            nc.gpsimd.memset(tri2[:], 1.0)
            for a in range(2):
                nc.gpsimd.affine_select(
                    out=tri2[:, a, :],
                    in_=tri2[:, a, :],
                    compare_op=mybir.AluOpType.is_ge,
                    fill=0.0,
                    base=0,
                    pattern=[[1, P]],
                    channel_multiplier=-1,
                )

            QT = persist.tile([P, MCH, T], BF16, tag="QT")
            KT = persist.tile([P, MCH, T], BF16, tag="KT")
            VE = persist.tile([P, NTB, HLOC, D + 1], BF16, tag="VE")
            nc.vector.memset(VE[:, :, :, D : D + 1], 1.0)

            wts = {}

            def load_w(nm, src):
                wt = persist.tile([P, KCH, DLOC], BF16, tag=f"w{nm}",
                                  name=f"w{nm}")
                nc.sync.dma_start(wt[:], src[:, :, :])
                wts[nm] = wt

            x_tiles = {"q": {}, "kv": {}}

            def get_x(which, n):
                cache = x_tiles[which]
                if n not in cache:
                    pool = xqpool if which == "q" else xkpool
                    src = xq_v if which == "q" else xkv_v
                    xt = pool.tile([P, KCH, 512], BF16, tag="x",
                                   name=f"x{which}{n}")
                    nc.gpsimd.dma_start(xt[:], src[:, :, 512 * n : 512 * n + 512])
                    cache[n] = xt
                return cache[n]

            # ---- projection units (8 matmuls + 1 cast each) ----
            def unit_qk(nm, n, m):
                dst = QT if nm == "q" else KT
                xt = get_x("q" if nm == "q" else "kv", n)
                wt = wts[nm]
                ps = pj_ps.tile([P, 512], F32, tag="pj")
                for k in range(KCH):
                    nc.tensor.matmul(
                        ps[:],
                        wt[:, k, P * m : P * m + P],
                        xt[:, k, :],
                        start=(k == 0),
                        stop=(k == KCH - 1),
                    )
                nc.vector.tensor_copy(dst[:, m, 512 * n : 512 * n + 512], ps[:])

            def unit_v(n, r):
                i = 4 * n + r
                xt = get_x("kv", n)
                wt = wts["v"]
                ps = pj_ps.tile([P, 512], F32, tag="pj")
                for k in range(KCH):
                    nc.tensor.matmul(
                        ps[:],
                        xt[:, k, P * r : P * r + P],
                        wt[:, k, :],
                        start=(k == 0),
                        stop=(k == KCH - 1),
                    )
                nc.vector.tensor_copy(
                    VE[:, i, :, 0:D],
                    ps[:].rearrange("p (h d) -> p h d", h=HLOC),
                )

            def proj_units(n):
                # ordered by consumption deadline in chunk n: strip m of
                # QT/KT is first read at (n, h=2m); VE block r at (n, ~r+1)
                us = []
                for m in range(MCH):
                    us.append(lambda m=m: unit_qk("q", n, m))
                    us.append(lambda m=m: unit_qk("k", n, m))
                for r in range(4):
                    us.append(lambda r=r: unit_v(n, r))
                return us

            # ---- attention pieces ----
            def emit_qk_pair(c, h, kind, pidx):
                """Emit one QK pair: 2 matmuls -> 1 exp -> optional mask.

                Returns pv entries (pt, j, tile_off, tq_start, width).
                """
                spo, po = h // 2, D * (h % 2)
                q0 = 512 * c
                ps = qk_ps.tile([P, 1024], F32, tag="qk", name="qps")
                pt = ptpool.tile([P, 1024], BF16, tag="pt", name="pt")
                if kind == "off":
                    j0 = 2 * pidx
                    for t2 in range(2):
                        j = j0 + t2
                        nc.tensor.matmul(
                            ps[:, 512 * t2 : 512 * t2 + 512],
                            KT[po : po + D, spo, P * j : P * j + P],
                            QT[po : po + D, spo, q0 : q0 + 512],
                            start=True,
                            stop=True,
                        )
                    nc.scalar.activation(pt[:], ps[:], EXP, scale=0.125)
                    return [(pt, j0, 0, 0, 512), (pt, j0 + 1, 512, 0, 512)]
                if kind == "dA":
                    j0, j1 = 4 * c, 4 * c + 1
                    nc.tensor.matmul(
                        ps[:, 0:512],
                        KT[po : po + D, spo, P * j0 : P * j0 + P],
                        QT[po : po + D, spo, q0 : q0 + 512],
                        start=True,
                        stop=True,
                    )
                    nc.tensor.matmul(
                        ps[:, 512:896],
                        KT[po : po + D, spo, P * j1 : P * j1 + P],
                        QT[po : po + D, spo, q0 + 128 : q0 + 512],
                        start=True,
                        stop=True,
                    )
                    nc.scalar.activation(pt[:, 0:896], ps[:, 0:896], EXP,
                                         scale=0.125)
                    # mask regions: block j0 cols [0,128) at off 0; block j1
                    # cols [128,256) at off 512+0
                    mv = pt[:].rearrange("p (a b) -> p a b", a=2)[:, :, 0:P]
                    nc.vector.tensor_mul(mv, mv, tri2[:])
                    return [(pt, j0, 0, 0, 512), (pt, j1, 512, 128, 384)]
                # dB
                j2, j3 = 4 * c + 2, 4 * c + 3
                nc.tensor.matmul(
                    ps[:, 0:256],
                    KT[po : po + D, spo, P * j2 : P * j2 + P],
                    QT[po : po + D, spo, q0 + 256 : q0 + 512],
                    start=True,
                    stop=True,
                )
                nc.tensor.matmul(
                    ps[:, 256:384],
                    KT[po : po + D, spo, P * j3 : P * j3 + P],
                    QT[po : po + D, spo, q0 + 384 : q0 + 512],
                    start=True,
                    stop=True,
                )
                nc.scalar.activation(pt[:, 0:384], ps[:, 0:384], EXP,
                                     scale=0.125)
                # mask regions: block j2 cols [256,384) at off 0; block j3
                # cols [384,512) at off 256
                mv = pt[:].rearrange("p (a b) -> p a b", a=4)[:, 0:2, 0:P]
                nc.vector.tensor_mul(mv, mv, tri2[:])
                return [(pt, j2, 0, 256, 256), (pt, j3, 256, 384, 128)]

            # ---- per-step state ----
            osb_tiles = {}
            pend_pv = None  # (c, h, entries) awaiting PV in the next step
            pend_ep = None  # (c, h, ot) awaiting transpose+divide epilogue

            def emit_epilogue(c, h, ot):
                tp4 = tp_ps.tile([P, 4, 72], F32, tag="tp", name="tp4")
                for s4 in range(4):
                    nc.tensor.transpose(
                        tp4[:, s4, 0 : D + 1],
                        ot[:, P * s4 : P * s4 + P],
                        ident[0 : D + 1, 0 : D + 1],
                    )
                r4 = rpool.tile([P, 4], F32, tag="r", name="r4")
                nc.vector.reciprocal(r4[:], tp4[:, :, D])
                oc = osb_tiles[c]
                for s4 in range(4):
                    nc.vector.tensor_scalar_mul(
                        oc[:, s4, D * h : D * h + D],
                        tp4[:, s4, 0:D],
                        r4[:, s4 : s4 + 1],
                    )
                if h == HLOC - 1:
                    nc.sync.dma_start(out_v[c], oc[:])

            def emit_pv_all(c, h, entries, units):
                """PV blocks for (c,h) with proj units interleaved; then the
                PSUM->SBUF cast of the PV accumulator."""
                pv = pv_ps.tile([D + 1, 512], F32, tag="pv", name="pv")
                n = len(entries)
                for idx, (pt, j, off, st, w) in enumerate(entries):
                    if idx % 2 == 1 and units:
                        units.pop(0)()
                    nc.tensor.matmul(
                        pv[:, st : st + w],
                        VE[:, j, h, :],
                        pt[:, off : off + w],
                        start=(idx == 0),
                        stop=(idx == n - 1),
                    )
                while units:
                    units.pop(0)()
                ot = otpool.tile([D + 1, 512], F32, tag="ot", name="ot")
                nc.vector.tensor_copy(ot[:], pv[:])
                return ot

            # ---- prologue: slice-0 projections, DMA-order-optimized ----
            load_w("q", wq_v)
            for m in range(MCH):
                unit_qk("q", 0, m)
            load_w("k", wk_v)
            for m in range(MCH):
                unit_qk("k", 0, m)
            load_w("v", wv_v)
            for r in range(4):
                unit_v(0, r)

            # ---- main steps ----
            for c in range(TC):
                osb_tiles[c] = osb.tile([P, 4, 512], F32, tag="o",
                                        name=f"osb{c}")
                units = proj_units(c + 1) if c + 1 < TC else []
                for h in range(HLOC):
                    # 2 proj units per step keeps 12 units ahead of their
                    # chunk-(c+1) deadlines; force-drain at h == 7
                    quota = len(units) if h == HLOC - 1 else 2
                    if pend_ep is not None:
                        emit_epilogue(*pend_ep)
                        pend_ep = None
                    kinds = [("off", p) for p in range(2 * c)]
                    kinds += [("dA", 0), ("dB", 0)]
                    entries = []
                    for kind, pidx in kinds:
                        entries += emit_qk_pair(c, h, kind, pidx)
                        if len(entries) == 4 and units and quota:
                            units.pop(0)()
                            quota -= 1
                    if pend_pv is not None:
                        c1, h1, e1 = pend_pv
                        u = []
                        while units and quota:
                            u.append(units.pop(0))
                            quota -= 1
                        ot = emit_pv_all(c1, h1, e1, u)
                        pend_ep = (c1, h1, ot)
                    pend_pv = (c, h, entries)

            # ---- drain ----
            if pend_ep is not None:
                emit_epilogue(*pend_ep)
            c1, h1, e1 = pend_pv
            ot = emit_pv_all(c1, h1, e1, [])
            emit_epilogue(c1, h1, ot)

    nc.compile()
    return nc


_NC_CACHE = {}


def _get_nc(T):
    if T not in _NC_CACHE:
        _NC_CACHE[T] = build(T)
    return _NC_CACHE[T]


def kernel(inputs_q, inputs_kv, Wq, Wk, Wv):
    inputs_q = np.asarray(inputs_q, dtype=np.float32)
    inputs_kv = np.asarray(inputs_kv, dtype=np.float32)
    Wq = np.asarray(Wq, dtype=np.float32)
    Wk = np.asarray(Wk, dtype=np.float32)
    Wv = np.asarray(Wv, dtype=np.float32)
    T = inputs_q.shape[1]

    bf = ml_dtypes.bfloat16
    in_maps = []
    for c in range(N_CORES):
        b, g = c // 2, c % 2
        sl = slice(g * DLOC, (g + 1) * DLOC)
        in_maps.append(
            {
                "xqT": np.ascontiguousarray(inputs_q[b].T).astype(bf),
                "xkvT": np.ascontiguousarray(inputs_kv[b].T).astype(bf),
                "wqT": np.ascontiguousarray(Wq[sl].T).astype(bf),
                "wkT": np.ascontiguousarray(Wk[sl].T).astype(bf),
                "wvT": np.ascontiguousarray(Wv[sl].T).astype(bf),
            }
        )

    nc = _get_nc(T)
    trace = bool(int(os.environ.get("KERNEL_TRACE", "0")))
    res = run_bass_kernel_spmd(
        nc, in_maps, core_ids=list(range(N_CORES)), trace=trace
    )
    if trace:
        kernel.last_result = res

    full = np.empty((B, T, E), np.float32)
    for c in range(N_CORES):
        b, g = c // 2, c % 2
        full[b, :, g * DLOC : (g + 1) * DLOC] = res.results[c]["out"]
    return full
